# revision 25
# baseline (speedup 1.0000x reference)
"""GQA cross-attention kernel for 8 trn2 NeuronCores — v2.

Problem: q [2, 2048, 32, 128] fp32, kv [2, 2048, 2, 8, 128] fp32
         -> softmax(q @ k^T / sqrt(128)) @ v  -> [2, 2048, 32, 128]

Sharding: 64 (batch, head) units over 8 cores: core c gets batch c//4,
q-heads [8*(c%4), 8*(c%4)+8) and kv-heads [2*(c%4), 2*(c%4)+2).

Device layout (host pre-transposes, free):
  qT  [8, 128, 2048]  q head-major, D on partitions (bf16)
  kT  [2, 128, 2048]  k head-major, D on partitions (bf16)
  vt  [2, 128, 16*128] v tiled: vt[i, p, t*128+d] = v[t*128+p, d] (bf16)
  oT  [8, 128, 2048]  UNNORMALIZED output O^T per head (f32)
  lp  [8, 4, 4, 512]  exp-sum partials (4 col-tile positions); host
                      sums the 4 partials and divides oT by l.

Per-core stream of 512 "k-steps" (32 blocks x 16 k-tiles; block =
(head, 512-wide q block)).  Step s:
  MM1:  S^T slice = K_tile^T . Q_block  (bf16, into a [128, 1536] PSUM
        window tile; 3 steps per window, 2 window bufs = 6 banks)
  exp:  one ACTIVATE per window ([128, 1536] PSUM->SBUF bf16); larger
        tiles amortize ACT's ~310-cycle fixed overhead
  MM2:  O^T += V_tile^T . P  (bf16, PSUM accumulation; 1 bank), lagged
        LAG steps behind MM1 so exp latency never stalls the PE
  sums: every 4 steps a burst of 4 column-tiled (128x32) matmuls by
        ones at PSUM partitions 0/32/64/96 of a single l bank
Block tail: DVE evacuates o_ps and l_ps to SBUF, DMA to HBM; the
final combine (sum 4 partials, divide) happens on the host.
"""

import math

import numpy as np

import concourse.bass as bass
import concourse.mybir as mybir
import concourse.tile as tile
from concourse import bacc
from concourse.bass import _add_dep_helper
from concourse.bass_utils import run_bass_kernel_spmd

F32 = mybir.dt.float32
BF16 = mybir.dt.bfloat16
EXP = mybir.ActivationFunctionType.Exp

B, SQ, SK, H, HKV, D = 2, 2048, 2048, 32, 8, 128
N_CORES = 8
H_PER_CORE = H * B // N_CORES  # 8
KV_PER_CORE = HKV * B // N_CORES  # 2
SCALE = 1.0 / math.sqrt(D)
SQ_BLK = 512
WIN = 3  # k-steps per exp window -> [128, WIN*512] ACTIVATE
LAG = 9  # steps between MM1 emission and MM2 emission


def build_nc(n_heads=H_PER_CORE, n_kv=KV_PER_CORE, sq=SQ, sk=SK):
    """Build the SPMD Bass program (identical on all cores)."""
    heads_per_kv = n_heads // n_kv  # 4
    sk_tiles = sk // 128  # 16
    sq_blocks = sq // SQ_BLK  # 4
    n_blocks = n_heads * sq_blocks  # 32
    n_steps = n_blocks * sk_tiles  # 512
    n_wins = (n_steps + WIN - 1) // WIN

    nc = bacc.Bacc("TRN2", target_bir_lowering=False, debug=False)

    qT = nc.dram_tensor("qT", [n_heads, D, sq], BF16, kind="ExternalInput")
    kT = nc.dram_tensor("kT", [n_kv, D, sk], BF16, kind="ExternalInput")
    vt = nc.dram_tensor("vt", [n_kv, 128, sk_tiles * D], BF16, kind="ExternalInput")
    ones = nc.dram_tensor("ones", [128, 1], BF16, kind="ExternalInput")
    oT = nc.dram_tensor("oT", [n_heads, D, sq], F32, kind="ExternalOutput")
    lp = nc.dram_tensor(
        "lp", [n_heads, sq_blocks, 4, SQ_BLK], F32, kind="ExternalOutput"
    )

    with tile.TileContext(nc) as tc:
        with (
            tc.tile_pool(name="inp", bufs=1) as inp_pool,
            tc.tile_pool(name="ppool", bufs=12) as ppool,
            tc.tile_pool(name="outp", bufs=5) as outp,
            tc.tile_pool(name="lout", bufs=5) as lout,
            tc.tile_pool(name="wpsum", bufs=2, space="PSUM") as wpsum,
            tc.tile_pool(name="opsum", bufs=1, space="PSUM") as opsum,
            tc.tile_pool(name="lpsum", bufs=1, space="PSUM") as lpsum,
        ):
            ones_sb = inp_pool.tile([128, 1], BF16, tag="ones", name="ones_sb")
            nc.vector.memset(ones_sb[:], 1.0)
            # Dummy exp to trigger the ACT table-set load (~2.7us) during
            # the DMA ramp instead of before the first real exp.
            warm_sb = inp_pool.tile([128, 1], BF16, tag="warm", name="warm_sb")
            nc.scalar.activation(warm_sb[:], ones_sb[:], EXP, scale=SCALE)

            q_sb = [
                inp_pool.tile([D, sq], BF16, tag=f"q{h}", name=f"q_sb{h}")
                for h in range(n_heads)
            ]
            k_sb = [
                inp_pool.tile([D, sk], BF16, tag=f"k{g}", name=f"k_sb{g}")
                for g in range(n_kv)
            ]
            v_sb = [
                inp_pool.tile([128, sk_tiles * D], BF16, tag=f"v{g}", name=f"v_sb{g}")
                for g in range(n_kv)
            ]

            # First wave on the sync (HWDGE) ring, in need-order for block 0.
            # Each dma_start costs ~0.6us of sync-queue time regardless of
            # size, so chunks are 128 cols only for the very first tiles and
            # 512+ cols after.  Everything beyond (g=0 later heads, all of
            # g=1) is issued from the gpsimd SWDGE ring in parallel.
            def need_order_dma(dst, src, sizes):
                off = 0
                for csz in sizes:
                    nc.sync.dma_start(
                        dst[:, off : off + csz], src[:, off : off + csz]
                    )
                    off += csz

            # v0 rides the second HWDGE ring (ACT queue — idle during the
            # ramp) so k0 and v0 stream in parallel.
            nc.sync.dma_start(k_sb[0][:, 0:128], kT[0][:, 0:128])
            nc.scalar.dma_start(v_sb[0][:, 0:512], vt[0][:, 0:512])
            nc.sync.dma_start(q_sb[0][:, 0:512], qT[0][:, 0:512])
            nc.scalar.dma_start(v_sb[0][:, 512:1280], vt[0][:, 512:1280])
            need_order_dma(k_sb[0][:, 128:], kT[0][:, 128:], [384, 512, 1024])
            nc.scalar.dma_start(v_sb[0][:, 1280:2048], vt[0][:, 1280:2048])
            need_order_dma(q_sb[0][:, 512:], qT[0][:, 512:], [512, 512, 512])
            for h in range(1, heads_per_kv):
                need_order_dma(q_sb[h][:], qT[h][:], [1024, 1024])
            for g in range(1, n_kv):
                need_order_dma(k_sb[g][:], kT[g][:], [1024, 1024])
                for hh in range(heads_per_kv):
                    h = g * heads_per_kv + hh
                    nc.gpsimd.dma_start(q_sb[h][:], qT[h][:])
                need_order_dma(v_sb[g][:], vt[g][:], [1024, 1024])

            p_of_win = [None] * n_wins
            wtile_of_win = [None] * n_wins
            wtile = None
            state = {"o_ps": None}

            def step_hjt(s):
                blk, t = divmod(s, sk_tiles)
                h, j = divmod(blk, sq_blocks)
                return blk, h, j, t

            def emit_mm2(d):
                blk, h, j, t = step_hjt(d)
                g = h // heads_per_kv
                w, c = divmod(d, WIN)
                if t == 0:
                    state["o_ps"] = opsum.tile(
                        [128, SQ_BLK], F32, tag="o", name="o_ps"
                    )
                o_ps = state["o_ps"]
                nc.tensor.matmul(
                    o_ps[:],
                    v_sb[g][:, bass.ts(t, 128)],
                    p_of_win[w][:, bass.ts(c, SQ_BLK)],
                    start=(t == 0),
                    stop=(t == sk_tiles - 1),
                    skip_group_check=True,
                )

            def emit_block_tail(d):
                blk, h, j, t = step_hjt(d)
                o_ps = state["o_ps"]
                # single 16-matmul sum burst per block: one tiling-mode
                # round trip instead of four.
                l_ps = lpsum.tile([128, SQ_BLK], F32, tag="l", name="l_ps")
                for tu in range(sk_tiles):
                    u = tu % 4
                    k4 = tu // 4
                    du = d - (sk_tiles - 1) + tu
                    wu, cu = divmod(du, WIN)
                    nc.tensor.matmul(
                        l_ps[32 * u : 32 * u + 1, :],
                        ones_sb[:],
                        p_of_win[wu][:, bass.ts(cu, SQ_BLK)],
                        start=(k4 == 0),
                        stop=(k4 == 3),
                        tile_position=(0, 32 * u),
                        skip_group_check=True,
                    )
                l_sb = lout.tile([128, SQ_BLK], F32, tag="ls", name="l_sb")
                nc.vector.tensor_copy(l_sb[:], l_ps[:])
                nc.sync.dma_start(lp[h, j], l_sb[0:97:32, :])
                ot_sb = outp.tile([128, SQ_BLK], F32, tag="ot", name="ot_sb")
                nc.vector.tensor_copy(ot_sb[:], o_ps[:])
                last = blk == n_blocks - 1
                if last:
                    # ACT is done by now; split the final output DMA
                    # across both HWDGE rings to shorten the drain.
                    half = SQ_BLK // 2
                    j0 = j * SQ_BLK
                    nc.sync.dma_start(
                        oT[h, :, j0 : j0 + half], ot_sb[:, :half]
                    )
                    nc.scalar.dma_start(
                        oT[h, :, j0 + half : j0 + SQ_BLK], ot_sb[:, half:]
                    )
                else:
                    nc.sync.dma_start(oT[h, :, bass.ts(j, SQ_BLK)], ot_sb[:])

            for s in range(n_steps + LAG):
                d = s - LAG
                if s < n_steps:
                    blk, h, j, t = step_hjt(s)
                    g = h // heads_per_kv
                    w, c = divmod(s, WIN)
                    if c == 0:
                        wtile = wpsum.tile(
                            [128, WIN * SQ_BLK], F32, tag="w", name="w_ps"
                        )
                        wtile_of_win[w] = wtile
                    nc.tensor.matmul(
                        wtile[:, bass.ts(c, SQ_BLK)],
                        k_sb[g][:, bass.ts(t, 128)],
                        q_sb[h][:, bass.ts(j, SQ_BLK)],
                        start=True,
                        stop=True,
                    )
                    if c == WIN - 1 or s == n_steps - 1:
                        width = (c + 1) * SQ_BLK
                        ptile = ppool.tile(
                            [128, WIN * SQ_BLK], BF16, tag="p", name="p_sb"
                        )
                        nc.scalar.activation(
                            ptile[:, :width], wtile[:, :width], EXP, scale=SCALE
                        )
                        p_of_win[w] = ptile
                if d >= 0:
                    emit_mm2(d)
                    if d % sk_tiles == sk_tiles - 1:
                        emit_block_tail(d)

    nc.compile()
    return nc


_NC_CACHE = {}


def _get_nc():
    if "nc" not in _NC_CACHE:
        _NC_CACHE["nc"] = build_nc()
    return _NC_CACHE["nc"]


def make_in_maps(q, kv):
    import ml_dtypes

    q = np.asarray(q)
    kv = np.asarray(kv)
    k = kv[:, :, 0]  # [B, Sk, Hkv, D]
    v = kv[:, :, 1]  # [B, Sk, Hkv, D]
    qT_all = np.ascontiguousarray(
        q.transpose(0, 2, 3, 1).astype(ml_dtypes.bfloat16)
    )  # [B, H, D, Sq]
    kT_all = np.ascontiguousarray(
        k.transpose(0, 2, 3, 1).astype(ml_dtypes.bfloat16)
    )  # [B, Hkv, D, Sk]
    # vt[b, hkv, p, t, d] = v[b, t*128 + p, hkv, d]
    vt_all = np.ascontiguousarray(
        v.reshape(B, SK // 128, 128, HKV, D)
        .transpose(0, 3, 2, 1, 4)
        .astype(ml_dtypes.bfloat16)
    ).reshape(B, HKV, 128, (SK // 128) * D)
    ones = np.ones((128, 1), ml_dtypes.bfloat16)

    in_maps = []
    for c in range(N_CORES):
        b = c // (N_CORES // B)
        part = c % (N_CORES // B)
        h0 = part * H_PER_CORE
        g0 = part * KV_PER_CORE
        in_maps.append(
            {
                "qT": qT_all[b, h0 : h0 + H_PER_CORE],
                "kT": kT_all[b, g0 : g0 + KV_PER_CORE],
                "vt": vt_all[b, g0 : g0 + KV_PER_CORE],
                "ones": ones,
            }
        )
    return in_maps


def gather_output(results):
    out = np.empty((B, SQ, H, D), np.float32)
    for c in range(N_CORES):
        b = c // (N_CORES // B)
        part = c % (N_CORES // B)
        h0 = part * H_PER_CORE
        oTc = results[c]["oT"]  # [8, 128, 2048] unnormalized O^T
        lpc = results[c]["lp"]  # [8, 4, 4, 512] exp-sum partials
        l = lpc.sum(axis=2).reshape(H_PER_CORE, SQ)  # [8, 2048]
        o = oTc / l[:, None, :]
        out[b, :, h0 : h0 + H_PER_CORE, :] = o.transpose(2, 0, 1)
    return out


def run(q, kv, trace=False, **kwargs):
    nc = _get_nc()
    in_maps = make_in_maps(q, kv)
    last_err = None
    for _attempt in range(3):
        try:
            res = run_bass_kernel_spmd(
                nc, in_maps, core_ids=list(range(N_CORES)), trace=trace, **kwargs
            )
            return gather_output(res.results), res
        except Exception as e:  # transient NRT device wedge: retry
            last_err = e
            import time

            time.sleep(5)
    raise last_err


def kernel(q, kv):
    out, _ = run(q, kv, trace=False)
    return out


# revision 27
# speedup vs baseline: 1.1154x; 1.1154x over previous
"""GQA cross-attention kernel for 8 trn2 NeuronCores — v2.

Problem: q [2, 2048, 32, 128] fp32, kv [2, 2048, 2, 8, 128] fp32
         -> softmax(q @ k^T / sqrt(128)) @ v  -> [2, 2048, 32, 128]

Sharding: 64 (batch, head) units over 8 cores: core c gets batch c//4,
q-heads [8*(c%4), 8*(c%4)+8) and kv-heads [2*(c%4), 2*(c%4)+2).

Device layout (host pre-transposes, free):
  qT  [8, 128, 2048]  q head-major, D on partitions (bf16)
  kT  [2, 128, 2048]  k head-major, D on partitions (bf16)
  vt  [2, 128, 16*128] v tiled: vt[i, p, t*128+d] = v[t*128+p, d] (bf16)
  oT  [8, 128, 2048]  UNNORMALIZED output O^T per head (f32)
  lp  [8, 4, 4, 512]  exp-sum partials (4 col-tile positions); host
                      sums the 4 partials and divides oT by l.

Per-core stream of 512 "k-steps" (32 blocks x 16 k-tiles; block =
(head, 512-wide q block)).  Step s:
  MM1:  S^T slice = K_tile^T . Q_block  (bf16, into a [128, 1536] PSUM
        window tile; 3 steps per window, 2 window bufs = 6 banks)
  exp:  one ACTIVATE per window ([128, 1536] PSUM->SBUF bf16); larger
        tiles amortize ACT's ~310-cycle fixed overhead
  MM2:  O^T += V_tile^T . P  (bf16, PSUM accumulation; 1 bank), lagged
        LAG steps behind MM1 so exp latency never stalls the PE
  sums: every 4 steps a burst of 4 column-tiled (128x32) matmuls by
        ones at PSUM partitions 0/32/64/96 of a single l bank
Block tail: DVE evacuates o_ps and l_ps to SBUF, DMA to HBM; the
final combine (sum 4 partials, divide) happens on the host.
"""

import math

import numpy as np

import concourse.bass as bass
import concourse.mybir as mybir
import concourse.tile as tile
from concourse import bacc
from concourse.bass import _add_dep_helper
from concourse.bass_utils import run_bass_kernel_spmd

F32 = mybir.dt.float32
BF16 = mybir.dt.bfloat16
EXP = mybir.ActivationFunctionType.Exp

B, SQ, SK, H, HKV, D = 2, 2048, 2048, 32, 8, 128
N_CORES = 8
H_PER_CORE = H * B // N_CORES  # 8
KV_PER_CORE = HKV * B // N_CORES  # 2
SCALE = 1.0 / math.sqrt(D)
SQ_BLK = 512
WIN = 3  # k-steps per exp window -> [128, WIN*512] ACTIVATE
LAG = 9  # steps between MM1 emission and MM2 emission


def build_nc(n_heads=H_PER_CORE, n_kv=KV_PER_CORE, sq=SQ, sk=SK):
    """Build the SPMD Bass program (identical on all cores)."""
    heads_per_kv = n_heads // n_kv  # 4
    sk_tiles = sk // 128  # 16
    sq_blocks = sq // SQ_BLK  # 4
    n_blocks = n_heads * sq_blocks  # 32
    n_steps = n_blocks * sk_tiles  # 512
    n_wins = (n_steps + WIN - 1) // WIN

    nc = bacc.Bacc("TRN2", target_bir_lowering=False, debug=False)

    qT = nc.dram_tensor("qT", [n_heads, D, sq], BF16, kind="ExternalInput")
    kT = nc.dram_tensor("kT", [n_kv, D, sk], BF16, kind="ExternalInput")
    vt = nc.dram_tensor("vt", [n_kv, 128, sk_tiles * D], BF16, kind="ExternalInput")
    ones = nc.dram_tensor("ones", [128, 1], BF16, kind="ExternalInput")
    oT = nc.dram_tensor("oT", [n_heads, D, sq], F32, kind="ExternalOutput")
    lp = nc.dram_tensor(
        "lp", [n_heads, sq_blocks, 4, SQ_BLK], F32, kind="ExternalOutput"
    )

    with tile.TileContext(nc) as tc:
        with (
            tc.tile_pool(name="inp", bufs=1) as inp_pool,
            tc.tile_pool(name="ppool", bufs=12) as ppool,
            tc.tile_pool(name="outp", bufs=5) as outp,
            tc.tile_pool(name="lout", bufs=5) as lout,
            tc.tile_pool(name="wpsum", bufs=2, space="PSUM") as wpsum,
            tc.tile_pool(name="opsum", bufs=1, space="PSUM") as opsum,
            tc.tile_pool(name="lpsum", bufs=1, space="PSUM") as lpsum,
        ):
            ones_sb = inp_pool.tile([128, 1], BF16, tag="ones", name="ones_sb")
            nc.vector.memset(ones_sb[:], 1.0)
            # Dummy exp to trigger the ACT table-set load (~2.7us) during
            # the DMA ramp instead of before the first real exp.
            warm_sb = inp_pool.tile([128, 1], BF16, tag="warm", name="warm_sb")
            nc.scalar.activation(warm_sb[:], ones_sb[:], EXP, scale=SCALE)

            q_sb = [
                inp_pool.tile([D, sq], BF16, tag=f"q{h}", name=f"q_sb{h}")
                for h in range(n_heads)
            ]
            k_sb = [
                inp_pool.tile([D, sk], BF16, tag=f"k{g}", name=f"k_sb{g}")
                for g in range(n_kv)
            ]
            v_sb = [
                inp_pool.tile([128, sk_tiles * D], BF16, tag=f"v{g}", name=f"v_sb{g}")
                for g in range(n_kv)
            ]

            # First wave on the sync (HWDGE) ring, in need-order for block 0.
            # Each dma_start costs ~0.6us of sync-queue time regardless of
            # size, so chunks are 128 cols only for the very first tiles and
            # 512+ cols after.  Everything beyond (g=0 later heads, all of
            # g=1) is issued from the gpsimd SWDGE ring in parallel.
            def need_order_dma(dst, src, sizes):
                off = 0
                for csz in sizes:
                    nc.sync.dma_start(
                        dst[:, off : off + csz], src[:, off : off + csz]
                    )
                    off += csz

            nc.sync.dma_start(k_sb[0][:, 0:128], kT[0][:, 0:128])
            nc.sync.dma_start(q_sb[0][:, 0:512], qT[0][:, 0:512])
            nc.sync.dma_start(v_sb[0][:, 0:512], vt[0][:, 0:512])
            need_order_dma(k_sb[0][:, 128:], kT[0][:, 128:], [384, 512, 1024])
            need_order_dma(v_sb[0][:, 512:], vt[0][:, 512:], [512, 512, 512])
            need_order_dma(q_sb[0][:, 512:], qT[0][:, 512:], [512, 512, 512])
            for h in range(1, heads_per_kv):
                need_order_dma(q_sb[h][:], qT[h][:], [1024, 1024])
            for g in range(1, n_kv):
                need_order_dma(k_sb[g][:], kT[g][:], [1024, 1024])
                for hh in range(heads_per_kv):
                    h = g * heads_per_kv + hh
                    nc.gpsimd.dma_start(q_sb[h][:], qT[h][:])
                need_order_dma(v_sb[g][:], vt[g][:], [1024, 1024])

            p_of_win = [None] * n_wins
            wtile_of_win = [None] * n_wins
            wtile = None
            state = {"o_ps": None}

            def step_hjt(s):
                blk, t = divmod(s, sk_tiles)
                h, j = divmod(blk, sq_blocks)
                return blk, h, j, t

            def emit_mm2(d):
                blk, h, j, t = step_hjt(d)
                g = h // heads_per_kv
                w, c = divmod(d, WIN)
                if t == 0:
                    state["o_ps"] = opsum.tile(
                        [128, SQ_BLK], F32, tag="o", name="o_ps"
                    )
                o_ps = state["o_ps"]
                nc.tensor.matmul(
                    o_ps[:],
                    v_sb[g][:, bass.ts(t, 128)],
                    p_of_win[w][:, bass.ts(c, SQ_BLK)],
                    start=(t == 0),
                    stop=(t == sk_tiles - 1),
                    skip_group_check=True,
                )

            def emit_block_tail(d):
                blk, h, j, t = step_hjt(d)
                o_ps = state["o_ps"]
                # single 16-matmul sum burst per block: one tiling-mode
                # round trip instead of four.
                l_ps = lpsum.tile([128, SQ_BLK], F32, tag="l", name="l_ps")
                for tu in range(sk_tiles):
                    u = tu % 4
                    k4 = tu // 4
                    du = d - (sk_tiles - 1) + tu
                    wu, cu = divmod(du, WIN)
                    nc.tensor.matmul(
                        l_ps[32 * u : 32 * u + 1, :],
                        ones_sb[:],
                        p_of_win[wu][:, bass.ts(cu, SQ_BLK)],
                        start=(k4 == 0),
                        stop=(k4 == 3),
                        tile_position=(0, 32 * u),
                        skip_group_check=True,
                    )
                l_sb = lout.tile([128, SQ_BLK], F32, tag="ls", name="l_sb")
                nc.vector.tensor_copy(l_sb[:], l_ps[:])
                nc.sync.dma_start(lp[h, j], l_sb[0:97:32, :])
                ot_sb = outp.tile([128, SQ_BLK], F32, tag="ot", name="ot_sb")
                nc.vector.tensor_copy(ot_sb[:], o_ps[:])
                nc.sync.dma_start(oT[h, :, bass.ts(j, SQ_BLK)], ot_sb[:])

            for s in range(n_steps + LAG):
                d = s - LAG
                if s < n_steps:
                    blk, h, j, t = step_hjt(s)
                    g = h // heads_per_kv
                    w, c = divmod(s, WIN)
                    if c == 0:
                        wtile = wpsum.tile(
                            [128, WIN * SQ_BLK], F32, tag="w", name="w_ps"
                        )
                        wtile_of_win[w] = wtile
                    nc.tensor.matmul(
                        wtile[:, bass.ts(c, SQ_BLK)],
                        k_sb[g][:, bass.ts(t, 128)],
                        q_sb[h][:, bass.ts(j, SQ_BLK)],
                        start=True,
                        stop=True,
                    )
                    if c == WIN - 1 or s == n_steps - 1:
                        width = (c + 1) * SQ_BLK
                        ptile = ppool.tile(
                            [128, WIN * SQ_BLK], BF16, tag="p", name="p_sb"
                        )
                        nc.scalar.activation(
                            ptile[:, :width], wtile[:, :width], EXP, scale=SCALE
                        )
                        p_of_win[w] = ptile
                if d >= 0:
                    emit_mm2(d)
                    if d % sk_tiles == sk_tiles - 1:
                        emit_block_tail(d)

    nc.compile()
    return nc


_NC_CACHE = {}


def _get_nc():
    if "nc" not in _NC_CACHE:
        _NC_CACHE["nc"] = build_nc()
    return _NC_CACHE["nc"]


def make_in_maps(q, kv):
    import ml_dtypes

    q = np.asarray(q)
    kv = np.asarray(kv)
    k = kv[:, :, 0]  # [B, Sk, Hkv, D]
    v = kv[:, :, 1]  # [B, Sk, Hkv, D]
    qT_all = np.ascontiguousarray(
        q.transpose(0, 2, 3, 1).astype(ml_dtypes.bfloat16)
    )  # [B, H, D, Sq]
    kT_all = np.ascontiguousarray(
        k.transpose(0, 2, 3, 1).astype(ml_dtypes.bfloat16)
    )  # [B, Hkv, D, Sk]
    # vt[b, hkv, p, t, d] = v[b, t*128 + p, hkv, d]
    vt_all = np.ascontiguousarray(
        v.reshape(B, SK // 128, 128, HKV, D)
        .transpose(0, 3, 2, 1, 4)
        .astype(ml_dtypes.bfloat16)
    ).reshape(B, HKV, 128, (SK // 128) * D)
    ones = np.ones((128, 1), ml_dtypes.bfloat16)

    in_maps = []
    for c in range(N_CORES):
        b = c // (N_CORES // B)
        part = c % (N_CORES // B)
        h0 = part * H_PER_CORE
        g0 = part * KV_PER_CORE
        in_maps.append(
            {
                "qT": qT_all[b, h0 : h0 + H_PER_CORE],
                "kT": kT_all[b, g0 : g0 + KV_PER_CORE],
                "vt": vt_all[b, g0 : g0 + KV_PER_CORE],
                "ones": ones,
            }
        )
    return in_maps


def gather_output(results):
    out = np.empty((B, SQ, H, D), np.float32)
    for c in range(N_CORES):
        b = c // (N_CORES // B)
        part = c % (N_CORES // B)
        h0 = part * H_PER_CORE
        oTc = results[c]["oT"]  # [8, 128, 2048] unnormalized O^T
        lpc = results[c]["lp"]  # [8, 4, 4, 512] exp-sum partials
        l = lpc.sum(axis=2).reshape(H_PER_CORE, SQ)  # [8, 2048]
        o = oTc / l[:, None, :]
        out[b, :, h0 : h0 + H_PER_CORE, :] = o.transpose(2, 0, 1)
    return out


def run(q, kv, trace=False, **kwargs):
    nc = _get_nc()
    in_maps = make_in_maps(q, kv)
    last_err = None
    for _attempt in range(3):
        try:
            res = run_bass_kernel_spmd(
                nc, in_maps, core_ids=list(range(N_CORES)), trace=trace, **kwargs
            )
            return gather_output(res.results), res
        except Exception as e:  # transient NRT device wedge: retry
            last_err = e
            import time

            time.sleep(5)
    raise last_err


def kernel(q, kv):
    out, _ = run(q, kv, trace=False)
    return out


# revision 28
# speedup vs baseline: 1.1193x; 1.0034x over previous
"""GQA cross-attention kernel for 8 trn2 NeuronCores — v2.

Problem: q [2, 2048, 32, 128] fp32, kv [2, 2048, 2, 8, 128] fp32
         -> softmax(q @ k^T / sqrt(128)) @ v  -> [2, 2048, 32, 128]

Sharding: 64 (batch, head) units over 8 cores: core c gets batch c//4,
q-heads [8*(c%4), 8*(c%4)+8) and kv-heads [2*(c%4), 2*(c%4)+2).

Device layout (host pre-transposes, free):
  qT  [8, 128, 2048]  q head-major, D on partitions (bf16)
  kT  [2, 128, 2048]  k head-major, D on partitions (bf16)
  vt  [2, 128, 16*128] v tiled: vt[i, p, t*128+d] = v[t*128+p, d] (bf16)
  oT  [8, 128, 2048]  UNNORMALIZED output O^T per head (f32)
  lp  [8, 4, 4, 512]  exp-sum partials (4 col-tile positions); host
                      sums the 4 partials and divides oT by l.

Per-core stream of 512 "k-steps" (32 blocks x 16 k-tiles; block =
(head, 512-wide q block)).  Step s:
  MM1:  S^T slice = K_tile^T . Q_block  (bf16, into a [128, 1536] PSUM
        window tile; 3 steps per window, 2 window bufs = 6 banks)
  exp:  one ACTIVATE per window ([128, 1536] PSUM->SBUF bf16); larger
        tiles amortize ACT's ~310-cycle fixed overhead
  MM2:  O^T += V_tile^T . P  (bf16, PSUM accumulation; 1 bank), lagged
        LAG steps behind MM1 so exp latency never stalls the PE
  sums: every 4 steps a burst of 4 column-tiled (128x32) matmuls by
        ones at PSUM partitions 0/32/64/96 of a single l bank
Block tail: DVE evacuates o_ps and l_ps to SBUF, DMA to HBM; the
final combine (sum 4 partials, divide) happens on the host.
"""

import math

import numpy as np

import concourse.bass as bass
import concourse.mybir as mybir
import concourse.tile as tile
from concourse import bacc
from concourse.bass import _add_dep_helper
from concourse.bass_utils import run_bass_kernel_spmd

F32 = mybir.dt.float32
BF16 = mybir.dt.bfloat16
EXP = mybir.ActivationFunctionType.Exp

B, SQ, SK, H, HKV, D = 2, 2048, 2048, 32, 8, 128
N_CORES = 8
H_PER_CORE = H * B // N_CORES  # 8
KV_PER_CORE = HKV * B // N_CORES  # 2
SCALE = 1.0 / math.sqrt(D)
SQ_BLK = 512
WIN = 3  # k-steps per exp window -> [128, WIN*512] ACTIVATE
LAG = 9  # steps between MM1 emission and MM2 emission


def build_nc(n_heads=H_PER_CORE, n_kv=KV_PER_CORE, sq=SQ, sk=SK):
    """Build the SPMD Bass program (identical on all cores)."""
    heads_per_kv = n_heads // n_kv  # 4
    sk_tiles = sk // 128  # 16
    sq_blocks = sq // SQ_BLK  # 4
    n_blocks = n_heads * sq_blocks  # 32
    n_steps = n_blocks * sk_tiles  # 512
    n_wins = (n_steps + WIN - 1) // WIN

    nc = bacc.Bacc("TRN2", target_bir_lowering=False, debug=False)

    qT = nc.dram_tensor("qT", [n_heads, D, sq], BF16, kind="ExternalInput")
    kT = nc.dram_tensor("kT", [n_kv, D, sk], BF16, kind="ExternalInput")
    vt = nc.dram_tensor("vt", [n_kv, 128, sk_tiles * D], BF16, kind="ExternalInput")
    ones = nc.dram_tensor("ones", [128, 1], BF16, kind="ExternalInput")
    oT = nc.dram_tensor("oT", [n_heads, D, sq], F32, kind="ExternalOutput")
    lp = nc.dram_tensor(
        "lp", [n_heads, sq_blocks, 4, SQ_BLK], F32, kind="ExternalOutput"
    )

    with tile.TileContext(nc) as tc:
        with (
            tc.tile_pool(name="inp", bufs=1) as inp_pool,
            tc.tile_pool(name="ppool", bufs=16) as ppool,
            tc.tile_pool(name="outp", bufs=5) as outp,
            tc.tile_pool(name="lout", bufs=5) as lout,
            tc.tile_pool(name="wpsum", bufs=2, space="PSUM") as wpsum,
            tc.tile_pool(name="opsum", bufs=1, space="PSUM") as opsum,
            tc.tile_pool(name="lpsum", bufs=1, space="PSUM") as lpsum,
        ):
            ones_sb = inp_pool.tile([128, 1], BF16, tag="ones", name="ones_sb")
            nc.vector.memset(ones_sb[:], 1.0)
            # Dummy exp to trigger the ACT table-set load (~2.7us) during
            # the DMA ramp instead of before the first real exp.
            warm_sb = inp_pool.tile([128, 1], BF16, tag="warm", name="warm_sb")
            nc.scalar.activation(warm_sb[:], ones_sb[:], EXP, scale=SCALE)

            q_sb = [
                inp_pool.tile([D, sq], BF16, tag=f"q{h}", name=f"q_sb{h}")
                for h in range(n_heads)
            ]
            k_sb = [
                inp_pool.tile([D, sk], BF16, tag=f"k{g}", name=f"k_sb{g}")
                for g in range(n_kv)
            ]
            v_sb = [
                inp_pool.tile([128, sk_tiles * D], BF16, tag=f"v{g}", name=f"v_sb{g}")
                for g in range(n_kv)
            ]

            # First wave on the sync (HWDGE) ring, in need-order for block 0.
            # Each dma_start costs ~0.6us of sync-queue time regardless of
            # size, so chunks are 128 cols only for the very first tiles and
            # 512+ cols after.  Everything beyond (g=0 later heads, all of
            # g=1) is issued from the gpsimd SWDGE ring in parallel.
            def need_order_dma(dst, src, sizes):
                off = 0
                for csz in sizes:
                    nc.sync.dma_start(
                        dst[:, off : off + csz], src[:, off : off + csz]
                    )
                    off += csz

            nc.sync.dma_start(k_sb[0][:, 0:128], kT[0][:, 0:128])
            nc.sync.dma_start(q_sb[0][:, 0:512], qT[0][:, 0:512])
            nc.sync.dma_start(v_sb[0][:, 0:512], vt[0][:, 0:512])
            need_order_dma(k_sb[0][:, 128:], kT[0][:, 128:], [384, 512, 1024])
            need_order_dma(v_sb[0][:, 512:], vt[0][:, 512:], [512, 512, 512])
            need_order_dma(q_sb[0][:, 512:], qT[0][:, 512:], [512, 512, 512])
            for h in range(1, heads_per_kv):
                need_order_dma(q_sb[h][:], qT[h][:], [1024, 1024])
            for g in range(1, n_kv):
                need_order_dma(k_sb[g][:], kT[g][:], [1024, 1024])
                for hh in range(heads_per_kv):
                    h = g * heads_per_kv + hh
                    nc.gpsimd.dma_start(q_sb[h][:], qT[h][:])
                need_order_dma(v_sb[g][:], vt[g][:], [1024, 1024])

            p_of_win = [None] * n_wins
            wtile_of_win = [None] * n_wins
            wtile = None
            state = {"o_ps": None}

            def step_hjt(s):
                blk, t = divmod(s, sk_tiles)
                h, j = divmod(blk, sq_blocks)
                return blk, h, j, t

            def emit_mm2(d):
                blk, h, j, t = step_hjt(d)
                g = h // heads_per_kv
                w, c = divmod(d, WIN)
                if t == 0:
                    state["o_ps"] = opsum.tile(
                        [128, SQ_BLK], F32, tag="o", name="o_ps"
                    )
                o_ps = state["o_ps"]
                nc.tensor.matmul(
                    o_ps[:],
                    v_sb[g][:, bass.ts(t, 128)],
                    p_of_win[w][:, bass.ts(c, SQ_BLK)],
                    start=(t == 0),
                    stop=(t == sk_tiles - 1),
                    skip_group_check=True,
                )

            def emit_block_tail(d):
                blk, h, j, t = step_hjt(d)
                o_ps = state["o_ps"]
                # single 16-matmul sum burst per block: one tiling-mode
                # round trip instead of four.
                l_ps = lpsum.tile([128, SQ_BLK], F32, tag="l", name="l_ps")
                for tu in range(sk_tiles):
                    u = tu % 4
                    k4 = tu // 4
                    du = d - (sk_tiles - 1) + tu
                    wu, cu = divmod(du, WIN)
                    nc.tensor.matmul(
                        l_ps[32 * u : 32 * u + 1, :],
                        ones_sb[:],
                        p_of_win[wu][:, bass.ts(cu, SQ_BLK)],
                        start=(k4 == 0),
                        stop=(k4 == 3),
                        tile_position=(0, 32 * u),
                        skip_group_check=True,
                    )
                l_sb = lout.tile([128, SQ_BLK], F32, tag="ls", name="l_sb")
                nc.vector.tensor_copy(l_sb[:], l_ps[:])
                nc.sync.dma_start(lp[h, j], l_sb[0:97:32, :])
                ot_sb = outp.tile([128, SQ_BLK], F32, tag="ot", name="ot_sb")
                nc.vector.tensor_copy(ot_sb[:], o_ps[:])
                nc.sync.dma_start(oT[h, :, bass.ts(j, SQ_BLK)], ot_sb[:])

            for s in range(n_steps + LAG):
                d = s - LAG
                if s < n_steps:
                    blk, h, j, t = step_hjt(s)
                    g = h // heads_per_kv
                    w, c = divmod(s, WIN)
                    if c == 0:
                        wtile = wpsum.tile(
                            [128, WIN * SQ_BLK], F32, tag="w", name="w_ps"
                        )
                        wtile_of_win[w] = wtile
                    nc.tensor.matmul(
                        wtile[:, bass.ts(c, SQ_BLK)],
                        k_sb[g][:, bass.ts(t, 128)],
                        q_sb[h][:, bass.ts(j, SQ_BLK)],
                        start=True,
                        stop=True,
                    )
                    if c == WIN - 1 or s == n_steps - 1:
                        width = (c + 1) * SQ_BLK
                        ptile = ppool.tile(
                            [128, WIN * SQ_BLK], BF16, tag="p", name="p_sb"
                        )
                        nc.scalar.activation(
                            ptile[:, :width], wtile[:, :width], EXP, scale=SCALE
                        )
                        p_of_win[w] = ptile
                if d >= 0:
                    emit_mm2(d)
                    if d % sk_tiles == sk_tiles - 1:
                        emit_block_tail(d)

    nc.compile()
    return nc


_NC_CACHE = {}


def _get_nc():
    if "nc" not in _NC_CACHE:
        _NC_CACHE["nc"] = build_nc()
    return _NC_CACHE["nc"]


def make_in_maps(q, kv):
    import ml_dtypes

    q = np.asarray(q)
    kv = np.asarray(kv)
    k = kv[:, :, 0]  # [B, Sk, Hkv, D]
    v = kv[:, :, 1]  # [B, Sk, Hkv, D]
    qT_all = np.ascontiguousarray(
        q.transpose(0, 2, 3, 1).astype(ml_dtypes.bfloat16)
    )  # [B, H, D, Sq]
    kT_all = np.ascontiguousarray(
        k.transpose(0, 2, 3, 1).astype(ml_dtypes.bfloat16)
    )  # [B, Hkv, D, Sk]
    # vt[b, hkv, p, t, d] = v[b, t*128 + p, hkv, d]
    vt_all = np.ascontiguousarray(
        v.reshape(B, SK // 128, 128, HKV, D)
        .transpose(0, 3, 2, 1, 4)
        .astype(ml_dtypes.bfloat16)
    ).reshape(B, HKV, 128, (SK // 128) * D)
    ones = np.ones((128, 1), ml_dtypes.bfloat16)

    in_maps = []
    for c in range(N_CORES):
        b = c // (N_CORES // B)
        part = c % (N_CORES // B)
        h0 = part * H_PER_CORE
        g0 = part * KV_PER_CORE
        in_maps.append(
            {
                "qT": qT_all[b, h0 : h0 + H_PER_CORE],
                "kT": kT_all[b, g0 : g0 + KV_PER_CORE],
                "vt": vt_all[b, g0 : g0 + KV_PER_CORE],
                "ones": ones,
            }
        )
    return in_maps


def gather_output(results):
    out = np.empty((B, SQ, H, D), np.float32)
    for c in range(N_CORES):
        b = c // (N_CORES // B)
        part = c % (N_CORES // B)
        h0 = part * H_PER_CORE
        oTc = results[c]["oT"]  # [8, 128, 2048] unnormalized O^T
        lpc = results[c]["lp"]  # [8, 4, 4, 512] exp-sum partials
        l = lpc.sum(axis=2).reshape(H_PER_CORE, SQ)  # [8, 2048]
        o = oTc / l[:, None, :]
        out[b, :, h0 : h0 + H_PER_CORE, :] = o.transpose(2, 0, 1)
    return out


def run(q, kv, trace=False, **kwargs):
    nc = _get_nc()
    in_maps = make_in_maps(q, kv)
    last_err = None
    for _attempt in range(3):
        try:
            res = run_bass_kernel_spmd(
                nc, in_maps, core_ids=list(range(N_CORES)), trace=trace, **kwargs
            )
            return gather_output(res.results), res
        except Exception as e:  # transient NRT device wedge: retry
            last_err = e
            import time

            time.sleep(5)
    raise last_err


def kernel(q, kv):
    out, _ = run(q, kv, trace=False)
    return out
